# revision 38
# baseline (speedup 1.0000x reference)
"""Trainium2 Bass kernel for nn_L2GESRModule.

Reference computation:
    Fh_conv = Fh @ Wh + bh            (dead: only used via ones_like)
    ESF     = ones_like(Fh_conv)      -> gather indices are a fixed shift
    Y       = Fl @ Wl + bl
    out[b,i,j,:] = Y[b, min(i+1,H-1), min(j+1,W-1), :]

One 1x1-conv GEMM on Fl plus a static (+1,+1) clamped shift, data-parallel
over batch (1 image per core). Fh/Wh/bh are never loaded.

Transposed fp8 pipeline (rel-err gate is 2e-2; measured ~1.88e-2):
  - Host casts Fl to e3m4 fp8 and pre-transposes each image to X^T [CIN, P].
    Device computes Y^T = (X @ Wl)^T W-stationary: for cin-half kh /
    cout-half ch: psum[ch] += Wl[kh,ch]^T @ X^T[kh]. No on-chip
    transposes; X^T streams as the moving operand (N=512).
  - The device stores UNSHIFTED Y^T; the (+1,+1) clamped shift is a pure
    indexing op applied on the host during decode (same class of host work
    as the transpose/cast). This aligns evacs exactly with PSUM groups,
    removes all on-chip col-127 fixups, and decouples store gating from
    the shift.
  - W is the FIRST DMA on each ring (kh0/kh1 halves), host-side
    pre-arranged to [p, kh, n] so each partition row is one contiguous
    descriptor.
  - Loads ride THREE queues (both HWDGE rings + gpsimd SWDGE): the two
    rings alone top out ~290 GB/s combined, below the PE's ~300 GB/s
    streaming demand. Chunks are px-interleaved, both cin halves per
    chunk on ONE queue, so a matmul group depends on exactly one DMA.
  - PSUM tiles are [128, ch=2, 512] = 2 banks x 4 bufs = all 8 banks.
    Evacs alternate ACT/DVE; the last two groups split their cout halves
    across both engines so the tail chain is ~0.7us after the last MM.
  - 8 PE warmup matmuls on scratch data bridge the DMA preamble so the
    HAM clock-gate reaches 8/8 (2.4 GHz) with no PE idle gap; any PE gap
    mid-block re-throttles the clock to 1.2 GHz for ~3.4us.
  - Stores are held until the loads are off the HBM (per-NC HBM at
    ~330-360 GB/s combined is the wall) and ride the by-then-idle HWDGE
    rings, each chunk's two triggers paired across sync/scalar; SWDGE's
    ~1us Q7 emission and slow drain kept it off the store tail. The
    final chunk is 512 px so the post-last-evac drain is short.

Fixed envelope (not recoverable at kernel level): ~6.9us engine-boot/
preamble before the first DMA trigger and ~10us exit path (final store
HBM-write receipt, all-engine barrier, ~290 semaphore resets at ~65-115ns
each, notify). PE block floor is 128 MMs x 216ns = 27.6us (e3m4 runs at
bf16 speed; DoubleRow 2x needs e4m3/e5m2, which busts the 2e-2 rel-err
gate: e4m3 X/W quantization alone is ~2.6e-2 each).
"""

import numpy as np

import concourse.bacc as bacc
import concourse.mybir as mybir
from concourse import bass_utils, tile

B, H, W, CIN, COUT = 8, 128, 128, 256, 256
N_CORES = 8
P = H * W          # 16384 pixels per image
G = 512            # pixels per PSUM bank (fp32)
N_GROUPS = P // G  # 32
# store-chunk boundaries: small at the tail (short post-evac drain)
STORE_B = [0, 4096, 8192, 12288, 14336, 15360, 15872, 16384]
STORE_GATE_MIN = 16  # hold stores until loads are fully off the HBM
N_SWDGE_STORES = 0   # all stores ride the HWDGE rings (idle post-load);
                     # >0 would route that many early chunks via SWDGE
# px-interleaved load chunks round-robined over the three queues; small
# early chunks so the first matmul group is gated on ~128KB only.
CHUNKS = [256] * 2 + [512] * 5 + [1024] * 13
WARMUP_MM = 8
f16 = mybir.dt.float16
f32 = mybir.dt.float32
f8 = mybir.dt.float8e3  # e3m4: 4 mantissa bits, rel-err ~1.9e-2 end to end


def build_nc():
    n_store = len(STORE_B) - 1
    # store chunk k only needs the evacs covering its own pixel range
    store_gate = [
        max(STORE_B[k + 1] // G - 1, min(STORE_GATE_MIN + 2 * k, 31))
        for k in range(n_store)
    ]  # [16, 18, 23, 27, 29, 30, 31]
    assert store_gate == sorted(store_gate)
    starts = np.cumsum([0] + CHUNKS).tolist()

    nc = bacc.Bacc("TRN2", target_bir_lowering=False, debug=False)
    XT = nc.dram_tensor("XT", [2, 128, P], f8, kind="ExternalInput").ap()
    WT = nc.dram_tensor("WT", [128, 2, COUT], f16, kind="ExternalInput").ap()
    OT = nc.dram_tensor("outT", [2, 128, P], f8, kind="ExternalOutput").ap()

    with tile.TileContext(nc) as tc:
        with (
            tc.tile_pool(name="consts", bufs=1) as consts,
            tc.tile_pool(name="ps", bufs=4, space="PSUM") as ps_pool,
        ):
            # PE warmup: keep the PE busy while W + chunk 0 land so the HAM
            # clock-gate warms with no idle gap. Data is garbage.
            scratch = consts.tile([128, G], f16)
            nc.vector.memset(scratch, 0.25)
            ps_warm = ps_pool.tile([128, 2, G], f32, tag="ps")
            for _ in range(WARMUP_MM):
                nc.tensor.matmul(
                    ps_warm[:, 0], scratch[:, 0:128], scratch, start=True, stop=True
                )
            # short (N=128) warmup tail so the cutover to real matmuls is
            # fine-grained: whenever W+c0 land, at most ~107ns of warmup
            # remains in front of the first real matmul
            for _ in range(4):
                nc.tensor.matmul(
                    ps_warm[:, 0, 0:128], scratch[:, 0:128], scratch[:, 0:128],
                    start=True, stop=True,
                )

            w_sb = consts.tile([128, 2, COUT], f16)
            xt = consts.tile([128, 2, P], f8)
            out_sb = consts.tile([128, 2, P], f8)

            # W first on both rings, split into ch-quarters balanced across
            # the rings so the first-matmul gate is only ~128KB per ring
            # (W quarters + a 256px first chunk) through boot-contended HBM.
            nc.sync.dma_start(w_sb[:, 0, 0:128], WT[:, 0, 0:128])
            nc.scalar.dma_start(w_sb[:, 0, 128:256], WT[:, 0, 128:256])
            nc.sync.dma_start(w_sb[:, 1, 0:128], WT[:, 1, 0:128])
            nc.scalar.dma_start(w_sb[:, 1, 128:256], WT[:, 1, 128:256])
            XTp = XT.rearrange("kh p px -> p kh px")
            load_eng = [nc.sync, nc.scalar, nc.gpsimd]
            for c in range(len(CHUNKS)):
                load_eng[c % 3].dma_start(
                    xt[:, :, starts[c] : starts[c + 1]],
                    XTp[:, :, starts[c] : starts[c + 1]],
                )

            def store(sc):
                # stores fire only after the loads are off the HBM, so they
                # ride the idle HWDGE rings
                base, hi = STORE_B[sc], STORE_B[sc + 1]
                if sc >= N_SWDGE_STORES:
                    # pair the two triggers across sync/scalar so the late
                    # (dependency-tight) stores never serialize on one engine
                    nc.sync.dma_start(OT[0, :, base:hi], out_sb[:, 0, base:hi])
                    nc.scalar.dma_start(OT[1, :, base:hi], out_sb[:, 1, base:hi])
                else:
                    nc.gpsimd.dma_start(OT[0, :, base:hi], out_sb[:, 0, base:hi])
                    nc.gpsimd.dma_start(OT[1, :, base:hi], out_sb[:, 1, base:hi])

            for g in range(N_GROUPS):
                px = g * G
                ps = ps_pool.tile([128, 2, G], f32, tag="ps")
                for ch in (0, 1):
                    for kh in (0, 1):
                        nc.tensor.matmul(
                            ps[:, ch],
                            w_sb[:, kh, ch * 128 : (ch + 1) * 128],
                            xt[:, kh, px : px + G],
                            start=(kh == 0),
                            stop=(kh == 1),
                        )
                # evacuate both cout halves in one op (even g -> ACT); the
                # last two groups split halves across ACT/DVE so neither
                # engine serializes the tail
                if g >= N_GROUPS - 2:
                    _evac(nc, nc.scalar, ps[:, 0], out_sb[:, 0, px : px + G])
                    _evac(nc, nc.vector, ps[:, 1], out_sb[:, 1, px : px + G])
                else:
                    eng = nc.scalar if g % 2 == 0 else nc.vector
                    _evac(nc, eng, ps, out_sb[:, :, px : px + G])
                while store_gate and store_gate[0] == g:
                    store_gate.pop(0)
                    store(n_store - len(store_gate) - 1)
            assert not store_gate

    nc.compile()
    return nc


def _evac(nc, eng, src, dst):
    if eng is nc.scalar:
        eng.copy(dst, src)
    else:
        eng.tensor_scalar_add(dst, src, 0.0)


_cache: dict = {}


def _get_nc():
    if "nc" not in _cache:
        _cache["nc"] = build_nc()
    return _cache["nc"]


def prepare_in_maps(Fl, Wl):
    import ml_dtypes

    Fl = np.asarray(Fl, dtype=np.float32)
    WT = np.asarray(Wl, dtype=np.float32).astype(np.float16).reshape(2, 128, COUT)
    WT = np.ascontiguousarray(WT.transpose(1, 0, 2))  # [p, kh, n]
    in_maps = []
    for b in range(B):
        # x2 pre-scale centers randn data in e3m4's normal range (max ~15.5);
        # the host divides the output by 2 during decode
        x = (Fl[b].reshape(P, CIN) * 2.0).astype(ml_dtypes.float8_e3m4)
        xt = np.ascontiguousarray(x.T)
        in_maps.append({"XT": xt.reshape(2, 128, P), "WT": WT})
    return in_maps


def assemble_output(results, bl):
    bl = np.asarray(bl, dtype=np.float32)
    # (+1,+1) clamped shift applied on the host: out[i,j] = Y[min(i+1,127),
    # min(j+1,127)]
    idx = np.minimum(np.arange(H) + 1, H - 1)
    outs = []
    for b in range(B):
        yt = np.asarray(results[b]["outT"]).reshape(COUT, P)
        arr = yt.T.astype(np.float32) * 0.5       # [P, COUT], undo x2 scale
        arr = arr.reshape(H, W, COUT)[idx][:, idx]
        if np.any(bl):
            arr += bl
        outs.append(arr)
    return np.stack(outs, axis=0)


def kernel(Fh, Fl, Wh, bh, Wl, bl):
    nc = _get_nc()
    in_maps = prepare_in_maps(Fl, Wl)
    res = bass_utils.run_bass_kernel_spmd(nc, in_maps, core_ids=list(range(N_CORES)))
    return assemble_output(res.results, bl)


# revision 39
# speedup vs baseline: 1.0361x; 1.0361x over previous
"""Trainium2 Bass kernel for nn_L2GESRModule.

Reference computation:
    Fh_conv = Fh @ Wh + bh            (dead: only used via ones_like)
    ESF     = ones_like(Fh_conv)      -> gather indices are a fixed shift
    Y       = Fl @ Wl + bl
    out[b,i,j,:] = Y[b, min(i+1,H-1), min(j+1,W-1), :]

One 1x1-conv GEMM on Fl plus a static (+1,+1) clamped shift, data-parallel
over batch (1 image per core). Fh/Wh/bh are never loaded.

Transposed fp8 pipeline (rel-err gate is 2e-2; measured ~1.88e-2):
  - Host casts Fl to e3m4 fp8 and pre-transposes each image to X^T [CIN, P].
    Device computes Y^T = (X @ Wl)^T W-stationary: for cin-half kh /
    cout-half ch: psum[ch] += Wl[kh,ch]^T @ X^T[kh]. No on-chip
    transposes; X^T streams as the moving operand (N=512).
  - The device stores UNSHIFTED Y^T; the (+1,+1) clamped shift is a pure
    indexing op applied on the host during decode (same class of host work
    as the transpose/cast). This aligns evacs exactly with PSUM groups,
    removes all on-chip col-127 fixups, and decouples store gating from
    the shift.
  - W is the FIRST DMA on each ring (kh0/kh1 halves), host-side
    pre-arranged to [p, kh, n] so each partition row is one contiguous
    descriptor.
  - Loads ride THREE queues (both HWDGE rings + gpsimd SWDGE): the two
    rings alone top out ~290 GB/s combined, below the PE's ~300 GB/s
    streaming demand. Chunks are px-interleaved, both cin halves per
    chunk on ONE queue, so a matmul group depends on exactly one DMA.
  - PSUM tiles are [128, ch=2, 512] = 2 banks x 4 bufs = all 8 banks.
    Evacs alternate ACT/DVE; the last two groups split their cout halves
    across both engines so the tail chain is ~0.7us after the last MM.
  - 8 PE warmup matmuls on scratch data bridge the DMA preamble so the
    HAM clock-gate reaches 8/8 (2.4 GHz) with no PE idle gap; any PE gap
    mid-block re-throttles the clock to 1.2 GHz for ~3.4us.
  - Stores are held until the loads are off the HBM (per-NC HBM at
    ~330-360 GB/s combined is the wall) and ride the by-then-idle HWDGE
    rings, each chunk's two triggers paired across sync/scalar; SWDGE's
    ~1us Q7 emission and slow drain kept it off the store tail. The
    final chunk is 512 px so the post-last-evac drain is short.

Fixed envelope (not recoverable at kernel level): ~6.9us engine-boot/
preamble before the first DMA trigger and ~10us exit path (final store
HBM-write receipt, all-engine barrier, ~290 semaphore resets at ~65-115ns
each, notify). PE block floor is 128 MMs x 216ns = 27.6us (e3m4 runs at
bf16 speed; DoubleRow 2x needs e4m3/e5m2, which busts the 2e-2 rel-err
gate: e4m3 X/W quantization alone is ~2.6e-2 each).
"""

import numpy as np

import concourse.bacc as bacc
import concourse.mybir as mybir
from concourse import bass_utils, tile

B, H, W, CIN, COUT = 8, 128, 128, 256, 256
N_CORES = 8
P = H * W          # 16384 pixels per image
G = 512            # pixels per PSUM bank (fp32)
N_GROUPS = P // G  # 32
# store-chunk boundaries: small at the tail (short post-evac drain)
STORE_B = [0, 4096, 8192, 12288, 14336, 15360, 15872, 16384]
STORE_GATE_MIN = 16  # hold stores until loads are fully off the HBM
N_SWDGE_STORES = 0   # all stores ride the HWDGE rings (idle post-load);
                     # >0 would route that many early chunks via SWDGE
# px-interleaved load chunks round-robined over the three queues; small
# early chunks so the first matmul group is gated on ~128KB only.
CHUNKS = [512] * 6 + [1024] * 13
WARMUP_MM = 8
f16 = mybir.dt.float16
f32 = mybir.dt.float32
f8 = mybir.dt.float8e3  # e3m4: 4 mantissa bits, rel-err ~1.9e-2 end to end


def build_nc():
    n_store = len(STORE_B) - 1
    # store chunk k only needs the evacs covering its own pixel range
    store_gate = [
        max(STORE_B[k + 1] // G - 1, min(STORE_GATE_MIN + 2 * k, 31))
        for k in range(n_store)
    ]  # [16, 18, 23, 27, 29, 30, 31]
    assert store_gate == sorted(store_gate)
    starts = np.cumsum([0] + CHUNKS).tolist()

    nc = bacc.Bacc("TRN2", target_bir_lowering=False, debug=False)
    XT = nc.dram_tensor("XT", [2, 128, P], f8, kind="ExternalInput").ap()
    WT = nc.dram_tensor("WT", [128, 2, COUT], f16, kind="ExternalInput").ap()
    OT = nc.dram_tensor("outT", [2, 128, P], f8, kind="ExternalOutput").ap()

    with tile.TileContext(nc) as tc:
        with (
            tc.tile_pool(name="consts", bufs=1) as consts,
            tc.tile_pool(name="ps", bufs=4, space="PSUM") as ps_pool,
        ):
            # PE warmup: keep the PE busy while W + chunk 0 land so the HAM
            # clock-gate warms with no idle gap. Data is garbage.
            scratch = consts.tile([128, G], f16)
            nc.vector.memset(scratch, 0.25)
            ps_warm = ps_pool.tile([128, 2, G], f32, tag="ps")
            for _ in range(WARMUP_MM):
                nc.tensor.matmul(
                    ps_warm[:, 0], scratch[:, 0:128], scratch, start=True, stop=True
                )
            # short (N=128) warmup tail so the cutover to real matmuls is
            # fine-grained: whenever W+c0 land, at most ~107ns of warmup
            # remains in front of the first real matmul
            for _ in range(4):
                nc.tensor.matmul(
                    ps_warm[:, 0, 0:128], scratch[:, 0:128], scratch[:, 0:128],
                    start=True, stop=True,
                )

            w_sb = consts.tile([128, 2, COUT], f16)
            xt = consts.tile([128, 2, P], f8)
            out_sb = consts.tile([128, 2, P], f8)

            # W first on both rings: contiguous 512B/partition halves
            # (fewer, larger early DMAs beat quarters: each extra DMA
            # boundary on a ring costs ~0.5-1us of completion overhead).
            nc.sync.dma_start(w_sb[:, 0], WT[:, 0])
            nc.scalar.dma_start(w_sb[:, 1], WT[:, 1])
            XTp = XT.rearrange("kh p px -> p kh px")
            load_eng = [nc.sync, nc.scalar, nc.gpsimd]
            for c in range(len(CHUNKS)):
                load_eng[c % 3].dma_start(
                    xt[:, :, starts[c] : starts[c + 1]],
                    XTp[:, :, starts[c] : starts[c + 1]],
                )

            def store(sc):
                # stores fire only after the loads are off the HBM, so they
                # ride the idle HWDGE rings
                base, hi = STORE_B[sc], STORE_B[sc + 1]
                if sc >= N_SWDGE_STORES:
                    # pair the two triggers across sync/scalar so the late
                    # (dependency-tight) stores never serialize on one engine
                    nc.sync.dma_start(OT[0, :, base:hi], out_sb[:, 0, base:hi])
                    nc.scalar.dma_start(OT[1, :, base:hi], out_sb[:, 1, base:hi])
                else:
                    nc.gpsimd.dma_start(OT[0, :, base:hi], out_sb[:, 0, base:hi])
                    nc.gpsimd.dma_start(OT[1, :, base:hi], out_sb[:, 1, base:hi])

            for g in range(N_GROUPS):
                px = g * G
                ps = ps_pool.tile([128, 2, G], f32, tag="ps")
                for ch in (0, 1):
                    for kh in (0, 1):
                        nc.tensor.matmul(
                            ps[:, ch],
                            w_sb[:, kh, ch * 128 : (ch + 1) * 128],
                            xt[:, kh, px : px + G],
                            start=(kh == 0),
                            stop=(kh == 1),
                        )
                # evacuate both cout halves in one op (even g -> ACT); the
                # last two groups split halves across ACT/DVE so neither
                # engine serializes the tail
                if g >= N_GROUPS - 2:
                    _evac(nc, nc.scalar, ps[:, 0], out_sb[:, 0, px : px + G])
                    _evac(nc, nc.vector, ps[:, 1], out_sb[:, 1, px : px + G])
                else:
                    eng = nc.scalar if g % 2 == 0 else nc.vector
                    _evac(nc, eng, ps, out_sb[:, :, px : px + G])
                while store_gate and store_gate[0] == g:
                    store_gate.pop(0)
                    store(n_store - len(store_gate) - 1)
            assert not store_gate

    nc.compile()
    return nc


def _evac(nc, eng, src, dst):
    if eng is nc.scalar:
        eng.copy(dst, src)
    else:
        eng.tensor_scalar_add(dst, src, 0.0)


_cache: dict = {}


def _get_nc():
    if "nc" not in _cache:
        _cache["nc"] = build_nc()
    return _cache["nc"]


def prepare_in_maps(Fl, Wl):
    import ml_dtypes

    Fl = np.asarray(Fl, dtype=np.float32)
    WT = np.asarray(Wl, dtype=np.float32).astype(np.float16).reshape(2, 128, COUT)
    WT = np.ascontiguousarray(WT.transpose(1, 0, 2))  # [p, kh, n]
    in_maps = []
    for b in range(B):
        # x2 pre-scale centers randn data in e3m4's normal range (max ~15.5);
        # the host divides the output by 2 during decode
        x = (Fl[b].reshape(P, CIN) * 2.0).astype(ml_dtypes.float8_e3m4)
        xt = np.ascontiguousarray(x.T)
        in_maps.append({"XT": xt.reshape(2, 128, P), "WT": WT})
    return in_maps


def assemble_output(results, bl):
    bl = np.asarray(bl, dtype=np.float32)
    # (+1,+1) clamped shift applied on the host: out[i,j] = Y[min(i+1,127),
    # min(j+1,127)]
    idx = np.minimum(np.arange(H) + 1, H - 1)
    outs = []
    for b in range(B):
        yt = np.asarray(results[b]["outT"]).reshape(COUT, P)
        arr = yt.T.astype(np.float32) * 0.5       # [P, COUT], undo x2 scale
        arr = arr.reshape(H, W, COUT)[idx][:, idx]
        if np.any(bl):
            arr += bl
        outs.append(arr)
    return np.stack(outs, axis=0)


def kernel(Fh, Fl, Wh, bh, Wl, bl):
    nc = _get_nc()
    in_maps = prepare_in_maps(Fl, Wl)
    res = bass_utils.run_bass_kernel_spmd(nc, in_maps, core_ids=list(range(N_CORES)))
    return assemble_output(res.results, bl)
